# revision 8
# baseline (speedup 1.0000x reference)
"""Trainium2 kernel for nn_LightningGNN (CNN node-encoder + 2x GCN + mean-pool
+ classifier) on 8 NeuronCores.

Strategy (all compute on device):
  - nodes sharded 6250/core, per-core in-degree-sorted permutation
  - CNN encoder: banded-Toeplitz conv-as-matmul in bf16
  - GCN layer: h@W + dinv scale on PE/DVE, AllGather of node features
    (node-major bf16, feature-duplicated to 256B rows), per-edge gather via
    SBUF-source dma_gather (transpose mode), padded-segment reduce on DVE
  - mean-pool: one-hot (1/cnt) matmul + AllReduce; classifier folded in
Falls back to a pure-numpy path if the device path fails.
"""

import os
import numpy as np
import ml_dtypes

bf16 = ml_dtypes.bfloat16

C = 8
N = 50000
T = 512
H = 64
G = 512
NPC = N // C           # 6250
NPAD = 6400            # 50 chunks of 128
NG = NPAD // 2         # 3200, group width (partitions 0:64 / 64:128)
SLOTS = C * NPAD       # 51200
HALF = 32768
CHUNK = 6144           # gather chunk slots (multiple of 128)
LAST_HW_EXEC_NS = None


# ================================================================ numpy path
def _conv1d_np(x, w, b, stride, pad):
    n, cin, L = x.shape
    cout, _, k = w.shape
    xp = np.pad(x, ((0, 0), (0, 0), (pad, pad)))
    Lo = (L + 2 * pad - k) // stride + 1
    out = np.zeros((n, cout, Lo), np.float32)
    for kk in range(k):
        sl = xp[:, :, kk:kk + stride * Lo:stride]
        out += np.einsum("ncl,oc->nol", sl, w[:, :, kk], optimize=True)
    return out + b[None, :, None]


def _encoder_numpy(x, w1, b1, w2, b2, w3, b3):
    h = x[:, None, :]
    h = np.maximum(_conv1d_np(h, w1, b1, 2, 3), 0.0)
    h = np.maximum(_conv1d_np(h, w2, b2, 2, 2), 0.0)
    h = np.maximum(_conv1d_np(h, w3, b3, 2, 2), 0.0)
    return h.mean(axis=-1).astype(np.float32)


def _gcn_tail(h, edge_index, batch, gW1, gb1, gW2, gb2, lW, lb):
    n = h.shape[0]
    src = edge_index[0].astype(np.int64)
    dst = edge_index[1].astype(np.int64)
    deg = np.bincount(dst, minlength=n).astype(np.float32) + 1.0
    dinv = 1.0 / np.sqrt(deg)
    order = np.argsort(dst, kind="stable")
    s_s, d_s = src[order], dst[order]
    seg_starts = np.flatnonzero(np.r_[True, d_s[1:] != d_s[:-1]])
    seg_ids = d_s[seg_starts]

    def layer(hin, W, b):
        hw = hin @ W
        hn = hw * dinv[:, None]
        msg = hn[s_s]
        sums = np.add.reduceat(msg, seg_starts, axis=0)
        agg = np.zeros_like(hw)
        agg[seg_ids] = sums
        agg = (agg + hn) * dinv[:, None]
        return np.maximum(agg + b[None, :], 0.0)

    h1 = layer(h, gW1, gb1)
    h2 = layer(h1, gW2, gb2)
    bt = batch.astype(np.int64)
    cnt = np.bincount(bt, minlength=G).astype(np.float32)
    bstarts = np.flatnonzero(np.r_[True, bt[1:] != bt[:-1]])
    bsums = np.add.reduceat(h2, bstarts, axis=0)
    pooled = np.zeros((G, h2.shape[1]), np.float32)
    pooled[bt[bstarts]] = bsums
    pooled /= np.maximum(cnt, 1.0)[:, None]
    return (pooled @ lW + lb).astype(np.float32)


def _numpy_full(inputs):
    enc = _encoder_numpy(
        np.asarray(inputs["x"], np.float32),
        np.asarray(inputs["w1"], np.float32), np.asarray(inputs["b1"], np.float32),
        np.asarray(inputs["w2"], np.float32), np.asarray(inputs["b2"], np.float32),
        np.asarray(inputs["w3"], np.float32), np.asarray(inputs["b3"], np.float32))
    return _gcn_tail(enc, np.asarray(inputs["edge_index"]),
                     np.asarray(inputs["batch"]),
                     np.asarray(inputs["gW1"], np.float32),
                     np.asarray(inputs["gb1"], np.float32),
                     np.asarray(inputs["gW2"], np.float32),
                     np.asarray(inputs["gb2"], np.float32),
                     np.asarray(inputs["lW"], np.float32),
                     np.asarray(inputs["lb"], np.float32))


# ========================================================== weight packing
def _pack_weights(w1, w2, w3, gW1, gW2, lW):
    """Pack all stationary operands into one [128, ncols] bf16 tensor.

    Returns (wpk [128, NW] f32->bf16 later, offsets dict, piece lists).
    Piece = (src_tile, bp, K, coloff); lhsT slice = wpk[bp:bp+K, co:co+128].
    """
    blocks = []
    cache = {}

    def add(mat):
        key = mat.tobytes()
        if key not in cache:
            cache[key] = len(blocks) * 128
            blocks.append(mat)
        return cache[key]

    # conv1: out block b = (8 t1s, 16 oc); window t in [16b-3, 16b+18)
    p1 = []
    for b in range(32):
        wlo, whi = max(0, 16 * b - 3), min(T, 16 * b + 18)
        pieces = []
        t0 = wlo
        while t0 < whi:
            tile = t0 // 128
            seg_end = min((tile + 1) * 128, whi)
            bp = ((t0 - tile * 128) // 32) * 32
            K = seg_end - tile * 128 - bp
            mat = np.zeros((128, 128), np.float32)
            for r in range(K):
                t = tile * 128 + bp + r
                if t < wlo or t >= whi:
                    continue
                for t1s in range(8):
                    k = t - 2 * (8 * b + t1s) + 3
                    if 0 <= k < 7:
                        for oc in range(16):
                            mat[bp + r, t1s * 16 + oc] = w1[oc, 0, k]
            pieces.append((tile, bp, K, add(mat)))
            t0 = seg_end
        p1.append(pieces)

    # conv2: out block a = (4 t2s, 32 oc); rows of y1 tile g: (t1-8g)*16+ic
    p2 = []
    for a in range(32):
        t1lo, t1hi = max(0, 8 * a - 2), min(256, 8 * a + 9)
        pieces = []
        for g in range(t1lo // 8, (t1hi - 1) // 8 + 1):
            seg_lo, seg_hi = max(t1lo, 8 * g), min(t1hi, 8 * g + 8)
            r_lo, r_hi = (seg_lo - 8 * g) * 16, (seg_hi - 8 * g) * 16
            bp = (r_lo // 32) * 32
            K = r_hi - bp
            mat = np.zeros((128, 128), np.float32)
            for t1 in range(seg_lo, seg_hi):
                for ic in range(16):
                    r = (t1 - 8 * g) * 16 + ic
                    for t2s in range(4):
                        k = t1 - 2 * (4 * a + t2s) + 2
                        if 0 <= k < 5:
                            for oc in range(32):
                                mat[r, t2s * 32 + oc] = w2[oc, ic, k]
            pieces.append((g, bp, K, add(mat)))
        p2.append(pieces)

    # conv3: out block a = (2 t3s, 64 oc); rows of y2 tile g: (t2-4g)*32+ic
    p3 = []
    for a in range(32):
        t2lo, t2hi = max(0, 4 * a - 2), min(128, 4 * a + 7)
        pieces = []
        for g in range(t2lo // 4, (t2hi - 1) // 4 + 1):
            seg_lo, seg_hi = max(t2lo, 4 * g), min(t2hi, 4 * g + 4)
            r_lo, r_hi = (seg_lo - 4 * g) * 32, (seg_hi - 4 * g) * 32
            bp = (r_lo // 32) * 32
            K = r_hi - bp
            mat = np.zeros((128, 128), np.float32)
            for t2 in range(seg_lo, seg_hi):
                for ic in range(32):
                    r = (t2 - 4 * g) * 32 + ic
                    for t3s in range(2):
                        k = t2 - 2 * (2 * a + t3s) + 2
                        if 0 <= k < 5:
                            for oc in range(64):
                                mat[r, t3s * 64 + oc] = w3[oc, ic, k]
            pieces.append((g, bp, K, add(mat)))
        p3.append(pieces)

    offs = {}
    # fold (mean over 64 t3): rows (t3s, oc) -> col oc (group 0) / 64+oc (g1)
    fold = np.zeros((128, 128), np.float32)
    for r in range(128):
        fold[r, r % 64] = 1.0 / 64.0
    offs["fold0"] = add(fold)
    fold2 = np.zeros((128, 128), np.float32)
    for r in range(128):
        fold2[r, 64 + (r % 64)] = 1.0 / 64.0
    offs["fold1"] = add(fold2)
    # identity (bf16) for PE transposes
    offs["ident"] = add(np.eye(128, dtype=np.float32))
    # gW blocks: rows 0:64 cols 0:64 = gW (group0); rows 64:128 cols 64:128
    for name, W in (("gW1", gW1), ("gW2", gW2)):
        mat = np.zeros((128, 128), np.float32)
        mat[0:64, 0:64] = W
        mat[64:128, 64:128] = W
        offs[name] = add(mat)
    # lW block: rows 0:64 cols 0:2 ; rows 64:128 cols 2:4
    mat = np.zeros((128, 128), np.float32)
    mat[0:64, 0:2] = lW
    mat[64:128, 2:4] = lW
    offs["lW"] = add(mat)

    wpk = np.concatenate(blocks, axis=1).astype(bf16)
    return wpk, offs, p1, p2, p3


# ======================================================== graph preprocessing
def _ceil_to(x, m):
    return ((x + m - 1) // m) * m


def _preprocess_graph(edge_index, batch):
    src = np.asarray(edge_index[0], np.int64)
    dst = np.asarray(edge_index[1], np.int64)
    deg = np.bincount(dst, minlength=N)
    dinv = (1.0 / np.sqrt(deg + 1.0)).astype(np.float32)

    nodes_of_pos = np.empty((C, NPC), np.int64)
    pos_of_node = np.empty(N, np.int64)
    for c in range(C):
        own = np.arange(c * NPC, (c + 1) * NPC)
        order = np.argsort(-deg[own], kind="stable")
        nodes_of_pos[c] = own[order]
        pos_of_node[own[order]] = np.arange(NPC)
    slot_of_node = (np.arange(N) // NPC) * NPAD + pos_of_node

    ecore = dst // NPC
    sslot_all = slot_of_node[src]
    dpos_all = pos_of_node[dst]

    # per-core, per-side (A: sslot<HALF, B: >=HALF) degree by position
    degS = np.zeros((2, C, NPC), np.int64)
    core_edges = []
    for c in range(C):
        m = ecore == c
        ss, dp = sslot_all[m], dpos_all[m]
        isB = (ss >= HALF).astype(np.int64)
        degS[0, c] = np.bincount(dp[isB == 0], minlength=NPC)
        degS[1, c] = np.bincount(dp[isB == 1], minlength=NPC)
        core_edges.append((ss, dp, isB))

    # capacity per position per side: >=4, mult of 4, monotone non-increasing
    caps = []
    for side in range(2):
        cp = degS[side].max(axis=0)
        cp = np.maximum(((cp + 3) // 4) * 4, 4)
        cp = np.maximum.accumulate(cp[::-1])[::-1]
        cp = np.concatenate([cp, np.full(NPAD - NPC, 4, np.int64)])  # pads
        caps.append(cp)

    ZA = NPC            # core 0 zero row -> slot 6250 (< HALF)
    ZB = 7 * NPAD + NPC - HALF  # core 7 zero row, B-relative

    # build chunk plan shared by all cores, streams in order:
    # (A,g0) (B,g0) (A,g1) (B,g1)  [combine g0 between]
    # chunk = dict(side, grp, L, idx_col0, reduces=[(off, n, cap, col0)])
    plan_chunks = []
    stream_meta = []  # (side, grp, positions list offsets) for idx fill
    idx_cols = 0      # global 16-wrapped column offset (in slots/16)
    for grp in range(2):
        for side in range(2):
            p0, p1_ = grp * NG, (grp + 1) * NG
            j = p0
            while j < p1_:
                # accumulate whole positions into a chunk
                L = 0
                reduces = []
                run_start = j
                run_cap = caps[side][j]
                nrun = 0
                jj = j
                while jj < p1_ and L + caps[side][jj] <= CHUNK:
                    cpj = caps[side][jj]
                    if cpj != run_cap:
                        reduces.append((L - nrun * run_cap, nrun, run_cap,
                                        run_start - p0))
                        run_start, run_cap, nrun = jj, cpj, 0
                    L += cpj
                    nrun += 1
                    jj += 1
                reduces.append((L - nrun * run_cap, nrun, run_cap,
                                run_start - p0))
                Lp = _ceil_to(L, 128)
                plan_chunks.append(dict(side=side, grp=grp, L=Lp,
                                        ic0=idx_cols, reduces=reduces,
                                        jlo=j, jhi=jj, used=L))
                idx_cols += Lp // 16
                j = jj
    TOT = idx_cols * 16

    # per-core idx arrays
    gidx = np.zeros((C, 16, idx_cols), np.int16)
    for c in range(C):
        ss, dp, isB = core_edges[c]
        for side in range(2):
            m = isB == side
            ssS, dpS = ss[m], dp[m]
            o = np.argsort(dpS, kind="stable")
            ssS, dpS = ssS[o], dpS[o]
            # cumcount within each dpos
            starts = np.zeros(NPC, np.int64)
            cnts = np.bincount(dpS, minlength=NPC)
            starts[1:] = np.cumsum(cnts)[:-1]
            cc = np.arange(len(dpS)) - starts[dpS]
            vals = (ssS - (HALF if side else 0)).astype(np.int16)
            zval = ZB if side else ZA
            # per-chunk fill
            for ch in plan_chunks:
                if ch["side"] != side:
                    continue
                base = np.full(ch["L"], zval, np.int16)
                # positions jlo..jhi with capacities; offsets within chunk
                jlo, jhi = ch["jlo"], ch["jhi"]
                cps = caps[side][jlo:jhi]
                offs_ = np.zeros(jhi - jlo, np.int64)
                offs_[1:] = np.cumsum(cps)[:-1]
                sel = (dpS >= jlo) & (dpS < jhi)
                base[offs_[dpS[sel] - jlo] + cc[sel]] = vals[sel]
                w = base.reshape(ch["L"] // 16, 16).T  # wrap: j at [j%16, j//16]
                gidx[c, :, ch["ic0"]:ch["ic0"] + ch["L"] // 16] = w
    gidx_full = np.tile(gidx, (1, 8, 1))  # replicate to 128 partitions

    # pooling one-hot (1/cnt), per core, [NPAD, G] bf16
    bt = np.asarray(batch, np.int64)
    cnt = np.bincount(bt, minlength=G).astype(np.float32)
    inv_cnt = 1.0 / np.maximum(cnt, 1.0)
    oh = np.zeros((C, NPAD, G), np.float32)
    for c in range(C):
        g_of_pos = bt[nodes_of_pos[c]]
        oh[c, np.arange(NPC), g_of_pos] = inv_cnt[g_of_pos]

    # dinv broadcast in h-layout [128, NG]
    dinvb = np.empty((C, 128, NG), np.float32)
    for c in range(C):
        dv = np.concatenate([dinv[nodes_of_pos[c]],
                             np.ones(NPAD - NPC, np.float32)])
        for grp in range(2):
            dinvb[c, 64 * grp:64 * grp + 64, :] = dv[grp * NG:(grp + 1) * NG]

    return dict(nodes_of_pos=nodes_of_pos, plan_chunks=plan_chunks,
                gidx=gidx_full, oh=oh.astype(bf16), dinvb=dinvb, TOT=TOT,
                idx_cols=idx_cols, dinv=dinv)


# ============================================================ device program
def _build_program(meta, wpk_cols, p1, p2, p3, offs, debug):
    import concourse.bass as bass
    import concourse.bacc as bacc
    import concourse.mybir as mybir
    from concourse.tile import TileContext

    dt = mybir.dt
    AF = mybir.ActivationFunctionType
    AL = mybir.AluOpType
    AX = mybir.AxisListType
    plan_chunks = meta["plan_chunks"]
    idx_cols = meta["idx_cols"]

    nc = bacc.Bacc(num_devices=C)
    x_in = nc.dram_tensor("x", [NPAD, T], dt.bfloat16, kind="ExternalInput")
    wpk_in = nc.dram_tensor("wpk", [128, wpk_cols], dt.bfloat16,
                            kind="ExternalInput")
    bias_in = nc.dram_tensor("biasv", [128, 8], dt.float32,
                             kind="ExternalInput")
    dinv_in = nc.dram_tensor("dinvb", [128, NG], dt.float32,
                             kind="ExternalInput")
    gidx_in = nc.dram_tensor("gidx", [128, idx_cols], dt.int16,
                             kind="ExternalInput")
    oh_in = nc.dram_tensor("oh", [NPAD, G], dt.bfloat16, kind="ExternalInput")
    out_t = nc.dram_tensor("out", [G, 2], dt.float32, kind="ExternalOutput")
    dbg = {}
    if debug:
        for nm in ("dbg_h0", "dbg_h1", "dbg_h2"):
            dbg[nm] = nc.dram_tensor(nm, [128, NG], dt.bfloat16,
                                     kind="ExternalOutput")

    IO = offs["ident"]

    with TileContext(nc) as tc:
        with (
            tc.tile_pool(name="persist", bufs=1) as pp,
            tc.tile_pool(name="dram", bufs=1, space="DRAM") as dr,
        ):
            wt = pp.tile([128, wpk_cols], dt.bfloat16, tag="wt")
            nc.sync.dma_start(wt[:, :], wpk_in[:, :])
            bt_ = pp.tile([128, 8], dt.float32, tag="bt")
            nc.sync.dma_start(bt_[:, :], bias_in[:, :])
            dvt = pp.tile([128, NG], dt.float32, tag="dvt")
            nc.sync.dma_start(dvt[:, :], dinv_in[:, :])
            hA = pp.tile([128, NG], dt.bfloat16, tag="hA")
            hB = pp.tile([128, NG], dt.bfloat16, tag="hB")
            hn = pp.tile([128, NG], dt.bfloat16, tag="hn")
            nm_all = pp.tile([128, 50 * 64], dt.bfloat16, tag="nm")

            agin = dr.tile([NPAD, 128], dt.bfloat16, tag="agin")
            agout = dr.tile([SLOTS, 128], dt.bfloat16, tag="agout")
            plin = dr.tile([128, 8], dt.float32, tag="plin")
            plout = dr.tile([128, 8], dt.float32, tag="plout")

            def ident_ap(b, p):
                return wt[b:b + p, IO + b:IO + b + p]

            # ---------------------------- encoder -----------------------------
            with (
                tc.tile_pool(name="enc", bufs=2) as ep,
                tc.tile_pool(name="ey", bufs=1) as yp,
                tc.tile_pool(name="ey3", bufs=3) as y3p,
                tc.tile_pool(name="eps", bufs=3, space="PSUM") as eps,
                tc.tile_pool(name="epe", bufs=1, space="PSUM") as pep,
                tc.tile_pool(name="etp", bufs=2, space="PSUM") as tps,
            ):
                nblocks = [(i * 512, 512) for i in range(12)] + [(6144, 256)]
                for p0, nb in nblocks:
                    xT = [ep.tile([128, 512], dt.bfloat16, tag=f"xT{k}")
                          for k in range(4)]
                    for j in range(nb // 128):
                        xs = ep.tile([128, T], dt.bfloat16, tag="xs")
                        nc.sync.dma_start(
                            xs[:, :], x_in[p0 + j * 128:p0 + (j + 1) * 128, :])
                        for k in range(4):
                            ptt = tps.tile([128, 128], dt.bfloat16, tag="ptt")
                            nc.tensor.transpose(
                                ptt[:, :], xs[:, 128 * k:128 * (k + 1)],
                                ident_ap(0, 128))
                            nc.vector.tensor_copy(
                                xT[k][:, j * 128:(j + 1) * 128], ptt[:, :])
                    y1 = [yp.tile([128, 512], dt.bfloat16, tag=f"y1_{b}")
                          for b in range(32)]
                    for b in range(32):
                        ps = eps.tile([128, 512], dt.float32, tag="cps")
                        for pi, (tile, bp, K, co) in enumerate(p1[b]):
                            nc.tensor.matmul(
                                ps[:, :nb], wt[bp:bp + K, co:co + 128],
                                xT[tile][bp:bp + K, :nb],
                                start=(pi == 0), stop=(pi == len(p1[b]) - 1))
                        nc.scalar.activation(y1[b][:, :nb], ps[:, :nb],
                                             AF.Relu, bias=bt_[:, 0:1])
                    y2 = [yp.tile([128, 512], dt.bfloat16, tag=f"y2_{a}")
                          for a in range(32)]
                    for a in range(32):
                        ps = eps.tile([128, 512], dt.float32, tag="cps")
                        for pi, (g, bp, K, co) in enumerate(p2[a]):
                            nc.tensor.matmul(
                                ps[:, :nb], wt[bp:bp + K, co:co + 128],
                                y1[g][bp:bp + K, :nb],
                                start=(pi == 0), stop=(pi == len(p2[a]) - 1))
                        nc.scalar.activation(y2[a][:, :nb], ps[:, :nb],
                                             AF.Relu, bias=bt_[:, 1:2])
                    pe = pep.tile([128, 512], dt.float32, tag="pe")
                    # which fold halves does this block need?
                    segs = []  # (grp, lo_local, hi_local)
                    if p0 < NG:
                        segs.append((0, 0, min(nb, NG - p0)))
                    if p0 + nb > NG:
                        segs.append((1, max(0, NG - p0), nb))
                    for a in range(32):
                        ps = eps.tile([128, 512], dt.float32, tag="cps")
                        for pi, (g, bp, K, co) in enumerate(p3[a]):
                            nc.tensor.matmul(
                                ps[:, :nb], wt[bp:bp + K, co:co + 128],
                                y2[g][bp:bp + K, :nb],
                                start=(pi == 0), stop=(pi == len(p3[a]) - 1))
                        y3 = y3p.tile([128, 512], dt.bfloat16, tag="y3")
                        nc.scalar.activation(y3[:, :nb], ps[:, :nb],
                                             AF.Relu, bias=bt_[:, 2:3])
                        for grp, lo, hi in segs:
                            nc.tensor.matmul(
                                pe[64 * grp:64 * grp + 64, lo:hi],
                                wt[0:128, offs[f"fold{grp}"] + 64 * grp:
                                   offs[f"fold{grp}"] + 64 * grp + 64],
                                y3[:, lo:hi],
                                start=(a == 0), stop=(a == 31))
                    for grp, lo, hi in segs:
                        d0 = p0 + lo - grp * NG
                        nc.vector.tensor_copy(
                            hA[64 * grp:64 * grp + 64, d0:d0 + hi - lo],
                            pe[64 * grp:64 * grp + 64, lo:hi])

            if debug:
                nc.sync.dma_start(dbg["dbg_h0"][:, :], hA[:, :])

            # ---------------------------- GCN layers --------------------------
            with (
                tc.tile_pool(name="gc", bufs=1) as gp,
                tc.tile_pool(name="gch", bufs=2) as gch,
                tc.tile_pool(name="gps", bufs=2, space="PSUM") as gps,
                tc.tile_pool(name="gtp", bufs=2, space="PSUM") as gtp,
                tc.tile_pool(name="gpl", bufs=1, space="PSUM") as gpl,
            ):
                gsrc = gp.tile([128, SLOTS], dt.bfloat16, tag="gsrc")
                aggA = gp.tile([128, NG], dt.float32, tag="aggA")
                aggB = gp.tile([128, NG], dt.float32, tag="aggB")

                def layer(h_in, h_out, Woff, bcol, dump):
                    # hn = (h_in @ W) * dinv   (feature-major, grouped)
                    for grp in range(2):
                        sl = slice(64 * grp, 64 * grp + 64)
                        for f0 in range(0, NG, 512):
                            fw = min(512, NG - f0)
                            ps = gps.tile([128, 512], dt.float32, tag="mps")
                            nc.tensor.matmul(
                                ps[sl, :fw],
                                wt[sl, Woff + 64 * grp:Woff + 64 * grp + 64],
                                h_in[sl, f0:f0 + fw], start=True, stop=True)
                            nc.vector.tensor_tensor(
                                hn[sl, f0:f0 + fw], ps[sl, :fw],
                                dvt[sl, f0:f0 + fw], AL.mult)
                    # node-major bf16 chunks -> nm_all
                    for ch in range(50):
                        grp, lc = ch // 25, ch % 25
                        sl = slice(64 * grp, 64 * grp + 64)
                        tp = gtp.tile([128, 64], dt.bfloat16, tag="tp")
                        nc.tensor.transpose(
                            tp[:, :], hn[sl, 128 * lc:128 * lc + 128],
                            ident_ap(64 * grp, 64))
                        nc.vector.tensor_copy(nm_all[:, 64 * ch:64 * ch + 64],
                                              tp[:, :])
                    nc.vector.memset(nm_all[106:128, 64 * 48:64 * 49], 0.0)
                    nc.vector.memset(nm_all[:, 64 * 49:64 * 50], 0.0)
                    # dup write to agin, allgather, readback
                    for d in range(2):
                        nc.sync.dma_start(
                            agin.rearrange("(c p) (d f) -> p c d f",
                                           p=128, d=2)[:, :, d, :],
                            nm_all[:, :].rearrange("p (c f) -> p c f", f=64))
                    nc.gpsimd.collective_compute(
                        "AllGather", AL.bypass,
                        replica_groups=[list(range(C))],
                        ins=[agin[:, :].opt()], outs=[agout[:, :].opt()])
                    nc.sync.dma_start(
                        gsrc[:, :].rearrange("p (r e) -> p r e", e=128),
                        agout.rearrange("(r p) e -> p r e", p=128))
                    nc.gpsimd.tensor_copy(nm_all[0:16, 0:1], gsrc[0:16, 0:1])

                    # gather + segment reduce, grouped by (grp, side)
                    for grp in range(2):
                        for ch in plan_chunks:
                            if ch["grp"] != grp:
                                continue
                            side = ch["side"]
                            agg = aggB if side else aggA
                            L = ch["L"]
                            idxt = gch.tile([128, CHUNK // 16], dt.int16,
                                            tag="idxt")
                            nc.sync.dma_start(
                                idxt[:, :L // 16],
                                gidx_in[:, ch["ic0"]:ch["ic0"] + L // 16])
                            msg = gch.tile([128, CHUNK], dt.bfloat16,
                                           tag="msg")
                            nc.gpsimd.tensor_copy(
                                msg[0:16, 0:1].bitcast(dt.int16),
                                idxt[0:16, 0:1])
                            nc.gpsimd.dma_gather(
                                out_ap=msg[:, :L].rearrange(
                                    "p (o l) -> p o l", o=1),
                                in_ap=(gsrc[:, HALF:SLOTS] if side
                                       else gsrc[:, 0:HALF]),
                                idxs_ap=idxt[:, :L // 16],
                                num_idxs=L, num_idxs_reg=L,
                                elem_size=128, transpose=True,
                                sbuf_tokens_per_rank=128,
                                sbuf_free_dim_per_rank=256)
                            for off, n, cap, col0 in ch["reduces"]:
                                nc.vector.tensor_reduce(
                                    agg[:, col0:col0 + n],
                                    msg[:, off:off + n * cap].rearrange(
                                        "p (n k) -> p n k", k=cap),
                                    AX.X, AL.add)
                        # combine group grp
                        sl = slice(64 * grp, 64 * grp + 64)
                        nc.vector.tensor_tensor(aggA[sl, :], aggA[sl, :],
                                                aggB[sl, :], AL.add)
                        nc.vector.tensor_tensor(aggA[sl, :], aggA[sl, :],
                                                hn[sl, :], AL.add)
                        nc.vector.tensor_tensor(aggA[sl, :], aggA[sl, :],
                                                dvt[sl, :], AL.mult)
                        nc.scalar.activation(h_out[sl, :], aggA[sl, :],
                                             AF.Relu, bias=bt_[sl, bcol:bcol + 1])
                    if debug:
                        nc.sync.dma_start(dbg[dump][:, :], h_out[:, :])

                layer(hA, hB, offs["gW1"], 3, "dbg_h1")
                layer(hB, hA, offs["gW2"], 4, "dbg_h2")

                # ------------------------- pooling ---------------------------
                # z = h2 @ lW  -> zsb (bf16, rows 0:2 grp0 / 64:66 grp1)
                zsb = hn  # reuse
                for grp in range(2):
                    sl = slice(64 * grp, 64 * grp + 64)
                    zsl = slice(64 * grp, 64 * grp + 2)
                    for f0 in range(0, NG, 512):
                        fw = min(512, NG - f0)
                        ps = gps.tile([128, 512], dt.float32, tag="mps")
                        nc.tensor.matmul(
                            ps[zsl, :fw],
                            wt[sl, offs["lW"] + 2 * grp:offs["lW"] + 2 * grp + 2],
                            hA[sl, f0:f0 + fw], start=True, stop=True)
                        nc.vector.tensor_copy(zsb[zsl, f0:f0 + fw],
                                              ps[zsl, :fw])
                ppool = [gpl.tile([128, 2], dt.float32, tag=f"pl{gc}")
                         for gc in range(4)]
                for chn in range(50):
                    grp, lc = chn // 25, chn % 25
                    zsl = slice(64 * grp, 64 * grp + 2)
                    tp = gtp.tile([128, 64], dt.bfloat16, tag="tp")
                    nc.tensor.transpose(
                        tp[:, 0:2], zsb[zsl, 128 * lc:128 * lc + 128],
                        ident_ap(64 * grp, 2))
                    zt = gch.tile([128, 2], dt.bfloat16, tag="zt")
                    nc.vector.tensor_copy(zt[:, :], tp[:, 0:2])
                    ohsb = gch.tile([128, G], dt.bfloat16, tag="ohsb")
                    nc.sync.dma_start(ohsb[:, :],
                                      oh_in[128 * chn:128 * chn + 128, :])
                    for gc in range(4):
                        nc.tensor.matmul(
                            ppool[gc][:, :], ohsb[:, 128 * gc:128 * gc + 128],
                            zt[:, :], start=(chn == 0), stop=(chn == 49))
                plsb = gp.tile([128, 8], dt.float32, tag="plsb")
                for gc in range(4):
                    nc.vector.tensor_copy(plsb[:, 2 * gc:2 * gc + 2],
                                          ppool[gc][:, :])
                nc.sync.dma_start(plin[:, :], plsb[:, :])
                nc.gpsimd.collective_compute(
                    "AllReduce", AL.add, replica_groups=[list(range(C))],
                    ins=[plin[:, :].opt()], outs=[plout[:, :].opt()])
                nc.sync.dma_start(plsb[:, :], plout[:, :])
                nc.sync.dma_start(
                    out_t.rearrange("(gc p) o -> p gc o", p=128),
                    plsb[:, :].rearrange("p (gc o) -> p gc o", o=2))
    nc.finalize()
    return nc


# ================================================================= entry
def _device_path(inputs, debug=False):
    from concourse.bass_utils import run_bass_kernel_spmd
    global LAST_HW_EXEC_NS

    x = np.asarray(inputs["x"], np.float32)
    ei = np.asarray(inputs["edge_index"])
    batch = np.asarray(inputs["batch"])
    w1 = np.asarray(inputs["w1"], np.float32)
    b1 = np.asarray(inputs["b1"], np.float32)
    w2 = np.asarray(inputs["w2"], np.float32)
    b2 = np.asarray(inputs["b2"], np.float32)
    w3 = np.asarray(inputs["w3"], np.float32)
    b3 = np.asarray(inputs["b3"], np.float32)
    gW1 = np.asarray(inputs["gW1"], np.float32)
    gb1 = np.asarray(inputs["gb1"], np.float32)
    gW2 = np.asarray(inputs["gW2"], np.float32)
    gb2 = np.asarray(inputs["gb2"], np.float32)
    lW = np.asarray(inputs["lW"], np.float32)
    lb = np.asarray(inputs["lb"], np.float32)

    wpk, offs, p1, p2, p3 = _pack_weights(w1, w2, w3, gW1, gW2, lW)
    meta = _preprocess_graph(ei, batch)

    # bias vectors: [128, 8] f32 per core (same all cores)
    biasv = np.zeros((128, 8), np.float32)
    p = np.arange(128)
    biasv[:, 0] = b1[p % 16]
    biasv[:, 1] = b2[p % 32]
    biasv[:, 2] = b3[p % 64]
    biasv[:, 3] = gb1[p % 64]
    biasv[:, 4] = gb2[p % 64]

    nc = _build_program(meta, wpk.shape[1], p1, p2, p3, offs, debug)

    in_maps = []
    for c in range(C):
        xp = np.zeros((NPAD, T), np.float32)
        xp[:NPC] = x[meta["nodes_of_pos"][c]]
        in_maps.append({
            "x": xp.astype(bf16),
            "wpk": wpk,
            "biasv": biasv,
            "dinvb": meta["dinvb"][c],
            "gidx": meta["gidx"][c],
            "oh": meta["oh"][c],
        })
    trace = os.environ.get("BASS_TRACE") == "1"
    res = run_bass_kernel_spmd(nc, in_maps, core_ids=list(range(C)),
                               trace=trace)
    if getattr(res, "exec_time_ns", None):
        LAST_HW_EXEC_NS = res.exec_time_ns
    out = np.asarray(res.results[0]["out"], np.float32) + lb[None, :]

    if debug:
        _debug_compare(res, meta, inputs)
    return out


def _debug_compare(res, meta, inputs):
    x = np.asarray(inputs["x"], np.float32)
    ei = np.asarray(inputs["edge_index"])
    enc = _encoder_numpy(x, np.asarray(inputs["w1"], np.float32),
                         np.asarray(inputs["b1"], np.float32),
                         np.asarray(inputs["w2"], np.float32),
                         np.asarray(inputs["b2"], np.float32),
                         np.asarray(inputs["w3"], np.float32),
                         np.asarray(inputs["b3"], np.float32))
    src, dst = ei[0].astype(np.int64), ei[1].astype(np.int64)
    deg = np.bincount(dst, minlength=N).astype(np.float32) + 1.0
    dinv = 1.0 / np.sqrt(deg)

    def layer_np(hin, W, b):
        hw = hin @ W
        hnv = hw * dinv[:, None]
        agg = np.zeros_like(hw)
        np.add.at(agg, dst, hnv[src])
        return np.maximum((agg + hnv) * dinv[:, None] + b[None, :], 0.0)

    h1 = layer_np(enc, np.asarray(inputs["gW1"], np.float32),
                  np.asarray(inputs["gb1"], np.float32))
    h2 = layer_np(h1, np.asarray(inputs["gW2"], np.float32),
                  np.asarray(inputs["gb2"], np.float32))
    for nm, ref in (("dbg_h0", enc), ("dbg_h1", h1), ("dbg_h2", h2)):
        for c in range(min(2, C)):
            got = np.asarray(res.results[c][nm]).astype(np.float32)
            # reconstruct [NPC, H] from [128, NG]
            rec = np.empty((NPAD, H), np.float32)
            for grp in range(2):
                rec[grp * NG:(grp + 1) * NG] = got[64 * grp:64 * grp + 64].T
            expc = ref[meta["nodes_of_pos"][c]]
            err = np.abs(rec[:NPC] - expc).max()
            scale = np.abs(expc).max() + 1e-9
            print(f"  {nm} core{c}: abs={err:.4e} rel={err / scale:.4e}")


def kernel(**inputs):
    debug = os.environ.get("KERNEL_DEBUG") == "1"
    if os.environ.get("KERNEL_NO_TRN") != "1":
        try:
            return _device_path(inputs, debug=debug)
        except Exception:
            import traceback
            traceback.print_exc()
    return _numpy_full(inputs)
